# revision 15
# baseline (speedup 1.0000x reference)
"""CIF (Continuous Integrate-and-Fire) segment-reduce kernel for Trainium2 (8 NeuronCores).

Structure of the problem (B=32, T=2000, H=512, L_OUT=250, threshold=0.95):

  * The scan over T is a recurrence ONLY in the scalar integrator driven by
    `alphas` [B,T] (256 KB).  It never touches `hidden`.  We replicate the
    reference's sequential fp32 arithmetic exactly on the host (same op
    order -> bit-identical fire decisions), which yields, for every step t,
    at most two (output-slot, weight) contributions:
      - no fire:  alpha_t             -> slot n_prev
      - fire:     1 - integrate_{t-1} -> slot n_prev   (emitted frame's last term)
                  alpha_t - dist_comp -> slot n_prev+1 (next frame's first term)
    where n_prev = number of fires before t.  Contributions to slots that
    never get emitted (>= min(#fires, L_OUT)) are dropped, matching the
    reference's gather/valid masking.

  * The heavy part, out[b,l] = sum_t W[b,l,t] * hidden[b,t], is a banded
    matmul (band drift is exactly 15.625 slots per 125-step chunk since
    sum(alphas) == 250; deviation is a Brownian bridge, sigma <~2 slots).
    It runs on the 8 NeuronCores, data-parallel over B (4 examples/core).
    Per example the 250 output slots live in two PSUM "panels" (banks) of
    128 slots; each matmul accumulates W_tile[125,128]^T @ h_tile[125,512]
    into the panel(s) its band intersects (blocks [0,875) -> panel 0,
    [1125,2000) -> panel 1, boundary block [875,1125) -> both panels with
    disjoint column halves, which the weight builder asserts).

  * DMA strategy (the whole game -- this kernel is HBM-bound).  The host
    pre-casts hidden to fp16 (identical rounding to the in-flight DMA cast
    an earlier revision used -- but the device then reads 8.2 MB instead
    of 16.4).  Measured queue->SDMA-lane mapping on this runtime: both
    HWDGE rings pin every DMA to lanes 0-4; the SWDGE ring splits each DMA
    into 5 equal contiguous descriptor runs on lanes {5k..5k+4} (mod 16)
    where k is the DMA's issue index -- confirmed to 3 decimals against
    per-lane byte counts.  So everything rides SWDGE as 16 input DMAs
    (W, b0, b1, b2 x 4 examples) whose issue order was chosen by brute
    force to minimize the max per-lane byte load (0.704 MB vs 0.656
    ideal) while staggering example readiness for the matmul pipeline;
    outputs follow on the same queue and drain behind the input stream.

  * PE: fp16 single-pass matmuls, N=512.  The PE_HAM clock gate halves the
    PE clock unless it sees ~3.4 us of sustained activity, and re-throttles
    after ~5 us idle; zero-value dummy matmuls warm it before the first
    example and bridge inter-example gaps so the real matmuls run at 2.4
    GHz.  Panel copies go vector (panel 0) / scalar (panel 1) in parallel;
    output returns as fp16 (host casts up; ~2e-4 extra rounding against a
    2e-2 budget).

Memory traffic per core ~ 16.4 MB hidden read + 2.3 MB W read + 1 MB out
write ~= 19.7 MB vs 358 GB/s HBM-per-NC and ~390 GB/s lane-aggregate
=> ~52 us stream floor + ~8 us preamble + ~5 us tail.
"""

import numpy as np

B, T, H = 32, 2000, 512
L_OUT = 250
N_CORES = 8
EX_PER_CORE = B // N_CORES      # 4
NCHUNK = 16                     # T-chunks per example
KC = T // NCHUNK                # 125 steps per chunk
LPAD = 256                      # padded slot axis (2 panels x 128)

# Hidden streams in 3 blocks per example; partition p of a block tile holds
# the S consecutive timesteps t = t0 + S*p + j, j<S (one contiguous S*2 KB
# HBM read per partition).  Each matmul contracts sub-chunk j = the 125
# strided steps {t0 + S*p + j}; the weight builder permutes W rows to
# match, so the sum is unchanged.  Slot position at step t is t/8 +- dev
# (Brownian bridge, sigma ~1.6 slots), so block [0,875) can only touch
# panel 0 and block [1125,2000) only panel 1 (11+ sigma margins, asserted);
# the boundary block [875,1125) hits both.
BLOCKS = [  # (t0, t1, S = steps per partition line, panels)
    (0, 875, 7, (0,)),
    (875, 1125, 2, (0, 1)),
    (1125, 2000, 7, (1,)),
]
# Matmul order: panel-0 b0 block, panel-1 b2 block, then the small boundary
# matmuls close both panels -- so when the boundary tile is the last DMA to
# land, only 4 short matmuls + the copies remain on the critical tail.
MMS = (
    [(0, j, 0) for j in range(7)]
    + [(2, j, 1) for j in range(7)]
    + [(1, 0, 0), (1, 1, 0), (1, 0, 1), (1, 1, 1)]
)
NMM = len(MMS)                  # 18

# Issue order of the 16 input DMAs (4 per example, example-grouped).  Slot
# k lands on SDMA lanes {5k..5k+4} mod 16; this order gives max-lane load
# 0.704 MB (ideal 0.656) and puts the last example's boundary tile in the
# final slot so the tail chain after the final landing is just 4 matmuls
# + the panel copies.
SLOT_PLAN = [
    ["b1", "W", "b0", "b2"],
    ["b0", "b2", "W", "b1"],
    ["b1", "W", "b0", "b2"],
    ["b0", "b2", "W", "b1"],
]

N_WARM = 6    # dummy matmuls before the first example (HAM warm-up)
N_KEEP = 2    # dummy matmuls between examples (bridge idle < ~5 us window)

_PROGRAM = None        # cached compiled Bass program
LAST_RESULT = None     # BassKernelResults of the most recent run (introspection)
RUN_KWARGS = {}        # extra kwargs for run_bass_kernel_spmd (e.g. trace=True)


def _host_scan_weights(alphas: np.ndarray):
    """Replicates the reference scan's fp32 arithmetic exactly.

    Returns (wa, Ai, wb, Bi, ntot): per-step primary weight/slot, secondary
    (fire-only) weight/slot, and total fires per row.
    """
    a = np.ascontiguousarray(alphas, dtype=np.float32)
    Bb, Tt = a.shape
    ONE = np.float32(1.0)
    TH = np.float32(0.95)
    integrate = np.zeros(Bb, np.float32)
    n = np.zeros(Bb, np.int32)
    wa = np.empty((Bb, Tt), np.float32)
    wb = np.zeros((Bb, Tt), np.float32)
    Ai = np.empty((Bb, Tt), np.int32)
    Bi = np.empty((Bb, Tt), np.int32)
    for t in range(Tt):
        al = a[:, t]
        dist = ONE - integrate          # distribution_completion (fp32)
        integ = integrate + al          # fp32, same single add as reference
        f = integ > TH
        cur = np.where(f, dist, al)
        wa[:, t] = cur
        Ai[:, t] = n                    # n_prev
        wb[:, t] = np.where(f, al - cur, np.float32(0.0))
        Bi[:, t] = n + 1
        n = n + f
        integrate = np.where(f, integ - ONE, integ)  # exact subtract (Sterbenz)
    return wa, Ai, wb, Bi, n


def _build_weight_windows(alphas: np.ndarray) -> np.ndarray:
    """Returns W [B, KC, NMM, 128] float16 panel weight tiles."""
    wa, Ai, wb, Bi, ntot = _host_scan_weights(alphas)
    lim = np.minimum(ntot, L_OUT)[:, None].astype(np.int32)
    wa = np.where(Ai < lim, wa, np.float32(0.0))
    wb = np.where(Bi < lim, wb, np.float32(0.0))

    Wd = np.zeros((B, T, LPAD), np.float32)
    bi = np.arange(B)[:, None]
    ti = np.arange(T)[None, :]
    Wd[bi, ti, np.minimum(Bi, LPAD - 1)] = wb
    Wd[bi, ti, np.minimum(Ai, LPAD - 1)] = wa

    # panel-coverage asserts: every block's band must be inside the union of
    # the panels it is assigned to.
    for bl, (t0, t1, S, panels) in enumerate(BLOCKS):
        if 0 not in panels and Wd[:, t0:t1, :128].any():
            raise AssertionError(f"block {bl} has panel-0 mass but no panel-0 matmul")
        if 1 not in panels and Wd[:, t0:t1, 128:].any():
            raise AssertionError(f"block {bl} has panel-1 mass but no panel-1 matmul")

    W = np.empty((B, KC, NMM, 128), np.float16)
    for i, (bl, j, p) in enumerate(MMS):
        t0, t1, S, _ = BLOCKS[bl]
        # [B, p(=partition), j, slot] with t = t0 + S*p + j
        blk = Wd[:, t0:t1, :].reshape(B, KC, S, LPAD)
        W[:, :, i, :] = blk[:, :, j, p * 128 : (p + 1) * 128]
    return np.ascontiguousarray(W)


def _build_program():
    """Builds + compiles the per-core Bass/Tile program (SPMD, shared)."""
    import concourse.bacc as bacc
    import concourse.mybir as mybir
    import concourse.tile as tile

    nc = bacc.Bacc(
        "TRN2",
        target_bir_lowering=False,
        debug=False,
        num_devices=N_CORES,
        dynamic_dma_scratch_size=65536,
    )
    f32 = mybir.dt.float32
    f16 = mybir.dt.float16

    hid = nc.dram_tensor(
        "hidden_sh", [EX_PER_CORE, T, H], f16, kind="ExternalInput"
    )
    wwin = nc.dram_tensor(
        "w_sh", [EX_PER_CORE, KC, NMM, 128], f16, kind="ExternalInput"
    )
    out = nc.dram_tensor(
        "out_sh", [EX_PER_CORE, L_OUT, H], f16, kind="ExternalOutput"
    )

    with tile.TileContext(nc) as tc:
        with (
            tc.tile_pool(name="hp0", bufs=4) as hpool0,    # b0 [125,7,H]
            tc.tile_pool(name="hp1", bufs=4) as hpool1,    # b1 [125,2,H]
            tc.tile_pool(name="hp2", bufs=4) as hpool2,    # b2 [125,7,H]
            tc.tile_pool(name="wp", bufs=4) as wpool,
            tc.tile_pool(name="dummy", bufs=2) as dpool,
            tc.tile_pool(name="ob", bufs=8) as opool,
            tc.tile_pool(name="psp", bufs=3, space="PSUM") as pspool,
            tc.tile_pool(name="pspd", bufs=1, space="PSUM") as pspool_d,
        ):
            # HAM warm-up fodder: zeroed operands, dedicated PSUM bank.
            dw = dpool.tile([KC, 128], f16)
            drh = dpool.tile([KC, H], f16)
            nc.vector.memset(dw[:], 0.0)
            nc.vector.memset(drh[:], 0.0)
            dps = pspool_d.tile([128, H], f32, tag="dummy")

            def dummy_mms(n):
                for _ in range(n):
                    nc.tensor.matmul(dps[:], dw[:], drh[:], start=True, stop=True)

            # ---- input DMAs, in exact rotation slot order ----
            hpools = {"b0": hpool0, "b1": hpool1, "b2": hpool2}
            htiles = [dict() for _ in range(EX_PER_CORE)]
            wtiles = [None] * EX_PER_CORE
            for e in range(EX_PER_CORE):
                for kind in SLOT_PLAN[e]:
                    if kind == "W":
                        wt = wpool.tile([KC, NMM, 128], f16)
                        nc.gpsimd.dma_start(wt[:], wwin[e])
                        wtiles[e] = wt
                    else:
                        bl = int(kind[1])
                        t0, t1, S, _ = BLOCKS[bl]
                        ht = hpools[kind].tile([KC, S, H], f16, name=kind)
                        src = hid[e, t0:t1, :].rearrange("(p j) h -> p j h", j=S)
                        nc.gpsimd.dma_start(ht[:], src)
                        htiles[e][kind] = ht

            def rhs(e, bl, j):
                return htiles[e][f"b{bl}"][:, j, :]

            # ---- matmul + copy-out pipeline ----
            last_i = {p: max(i for i, m in enumerate(MMS) if m[2] == p) for p in (0, 1)}
            dummy_mms(N_WARM)
            for e in range(EX_PER_CORE):
                if e:
                    dummy_mms(N_KEEP)
                wt = wtiles[e]
                panels = [
                    pspool.tile([128, H], f32, name=f"panel{p}", tag=f"panel{p}")
                    for p in range(2)
                ]
                first = [True, True]
                for i, (bl, j, p) in enumerate(MMS):
                    nc.tensor.matmul(
                        panels[p][:], wt[:, i, :], rhs(e, bl, j),
                        start=first[p], stop=(i == last_i[p]),
                    )
                    first[p] = False
                ob0 = opool.tile([128, H], f16)
                nc.vector.tensor_copy(ob0[:], panels[0][:])
                nc.gpsimd.dma_start(out[e, 0:128, :], ob0[:])
                ob1 = opool.tile([128, H], f16)
                nc.scalar.copy(ob1[0 : L_OUT - 128, :], panels[1][0 : L_OUT - 128, :])
                nc.gpsimd.dma_start(out[e, 128:L_OUT, :], ob1[0 : L_OUT - 128, :])
    nc.compile()
    return nc


def kernel(hidden: np.ndarray, alphas: np.ndarray) -> np.ndarray:
    global _PROGRAM, LAST_RESULT
    from concourse.bass_utils import run_bass_kernel_spmd

    hidden = np.ascontiguousarray(np.asarray(hidden), dtype=np.float32)
    alphas = np.ascontiguousarray(np.asarray(alphas), dtype=np.float32)
    assert hidden.shape == (B, T, H) and alphas.shape == (B, T)

    hidden16 = hidden.astype(np.float16)
    Wwin = _build_weight_windows(alphas)

    if _PROGRAM is None:
        _PROGRAM = _build_program()
    nc = _PROGRAM

    in_maps = [
        {
            "hidden_sh": hidden16[i * EX_PER_CORE : (i + 1) * EX_PER_CORE],
            "w_sh": Wwin[i * EX_PER_CORE : (i + 1) * EX_PER_CORE],
        }
        for i in range(N_CORES)
    ]
    res = run_bass_kernel_spmd(nc, in_maps, list(range(N_CORES)), **RUN_KWARGS)
    LAST_RESULT = res
    out16 = np.concatenate([r["out_sh"] for r in res.results], axis=0)
    return out16.astype(np.float32)


# revision 16
# speedup vs baseline: 1.1444x; 1.1444x over previous
"""CIF (Continuous Integrate-and-Fire) segment-reduce kernel for Trainium2 (8 NeuronCores).

Structure of the problem (B=32, T=2000, H=512, L_OUT=250, threshold=0.95):

  * The scan over T is a recurrence ONLY in the scalar integrator driven by
    `alphas` [B,T] (256 KB).  It never touches `hidden`.  We replicate the
    reference's sequential fp32 arithmetic exactly on the host (same op
    order -> bit-identical fire decisions), which yields, for every step t,
    at most two (output-slot, weight) contributions:
      - no fire:  alpha_t             -> slot n_prev
      - fire:     1 - integrate_{t-1} -> slot n_prev   (emitted frame's last term)
                  alpha_t - dist_comp -> slot n_prev+1 (next frame's first term)
    where n_prev = number of fires before t.  Contributions to slots that
    never get emitted (>= min(#fires, L_OUT)) are dropped, matching the
    reference's gather/valid masking.

  * The heavy part, out[b,l] = sum_t W[b,l,t] * hidden[b,t], is a banded
    matmul (band drift is exactly 15.625 slots per 125-step chunk since
    sum(alphas) == 250; deviation is a Brownian bridge, sigma <~2 slots).
    It runs on the 8 NeuronCores, data-parallel over B (4 examples/core).
    Per example the 250 output slots live in two PSUM "panels" (banks) of
    128 slots; each matmul accumulates W_tile[125,128]^T @ h_tile[125,512]
    into the panel(s) its band intersects (blocks [0,875) -> panel 0,
    [1125,2000) -> panel 1, boundary block [875,1125) -> both panels with
    disjoint column halves, asserted on the host).

  * The dense W tiles (2.3 MB/core) are NOT shipped: the host sends 72 KB
    of per-(row, matmul-tile) scan data (slot_a, w_a, slot_b, w_b as fp16;
    dead entries get slot 200) and the vector engine expands them in SBUF
    against a gpsimd iota ramp: W = (iota==sa)*wa + (iota==sb)*wb -- 5
    elementwise ops on [125, 18*128] fp16 per example (~4 us each).
    Integer slot values <= 255 are exact in fp16, so the compare is exact.

  * DMA strategy (the whole game -- this kernel is HBM/SBUF-port-bound;
    measured per-SDMA-lane SBUF-write rate is ~12 GB/s, aggregate ~190
    GB/s).  The host pre-casts hidden to fp16 (identical rounding to the
    in-flight DMA cast an earlier revision used, but the device then
    reads/writes 8.2 MB instead of 16.4).  Measured queue->lane mapping:
    both HWDGE rings pin every DMA to lanes 0-4; the SWDGE ring splits
    each DMA into 5 equal contiguous descriptor runs on lanes {5k..5k+4}
    (mod 16) where k is the DMA's issue index (confirmed to 3 decimals
    against per-lane byte counts).  Everything rides SWDGE as 13 input
    DMAs (scan-data + b0/b1/b2 x 4 examples) in a brute-forced issue
    order: max per-lane load 0.589 MB vs 0.516 ideal, example readiness
    staggered, and the last example's small boundary tile lands last so
    the post-stream tail is just 4 matmuls + the panel copies.  Outputs
    (fp16, host casts up) follow on the same queue and drain behind the
    input stream.

  * PE: fp16 single-pass matmuls, N=512.  The PE_HAM clock gate halves the
    PE clock unless it sees ~3.4 us of sustained activity; zero-value
    dummy matmuls warm it up front and bridge inter-example gaps so real
    matmuls run at 2.4 GHz (0.38 us vs 0.63 us measured).  Panel copies
    go vector (panel 0) / scalar (panel 1) in parallel.

Memory traffic per core ~ 8.3 MB in + 1 MB out against ~179 GB/s HBM and
~190 GB/s SBUF-write => ~50 us stream + ~8 us preamble + ~5 us tail.
"""

import numpy as np

B, T, H = 32, 2000, 512
L_OUT = 250
N_CORES = 8
EX_PER_CORE = B // N_CORES      # 4
NCHUNK = 16                     # T-chunks per example
KC = T // NCHUNK                # 125 steps per chunk
LPAD = 256                      # padded slot axis (2 panels x 128)
DEAD = 200.0                    # slot sentinel that never matches iota (0..127)

# Hidden streams in 3 blocks per example; partition p of a block tile holds
# the S consecutive timesteps t = t0 + S*p + j, j<S (one contiguous S KB
# fp16 HBM read per partition).  Each matmul contracts sub-chunk j = the
# 125 strided steps {t0 + S*p + j}; the scan-data builder permutes rows to
# match.  Slot position at step t is t/8 +- dev (Brownian bridge, sigma
# ~1.6 slots), so block [0,875) can only touch panel 0 and block
# [1125,2000) only panel 1 (11+ sigma margins, asserted); the boundary
# block [875,1125) hits both.
BLOCKS = [  # (t0, t1, S = steps per partition line, panels)
    (0, 875, 7, (0,)),
    (875, 1125, 2, (0, 1)),
    (1125, 2000, 7, (1,)),
]
# Matmul order: panel-0 b0 block, panel-1 b2 block, then the small boundary
# matmuls close both panels -- so when the boundary tile is the last DMA to
# land, only 4 short matmuls + the copies remain on the critical tail.
MMS = (
    [(0, j, 0) for j in range(7)]
    + [(2, j, 1) for j in range(7)]
    + [(1, 0, 0), (1, 1, 0), (1, 0, 1), (1, 1, 1)]
)
NMM = len(MMS)                  # 18

# Issue order of the 13 input DMAs.  Slot k lands on SDMA lanes {5k..5k+4}
# mod 16; this order gives max-lane load 0.589 MB (ideal 0.516), staggers
# example readiness, and puts ex3's boundary tile in the final slot.
# Entries: (example, kind); SD is the packed scan-data block for all 4.
SLOT_PLAN = [
    (0, "SD"),
    (0, "b1"), (0, "b0"), (0, "b2"),
    (1, "b0"), (1, "b2"),
    (2, "b0"), (2, "b2"),
    (1, "b1"), (2, "b1"),
    (3, "b0"), (3, "b2"),
    (3, "b1"),
]

N_WARM = 6    # dummy matmuls before the first example (HAM warm-up)
N_KEEP = 2    # dummy matmuls between examples (bridge idle < ~5 us window)

_PROGRAM = None        # cached compiled Bass program
LAST_RESULT = None     # BassKernelResults of the most recent run (introspection)
RUN_KWARGS = {}        # extra kwargs for run_bass_kernel_spmd (e.g. trace=True)


def _host_scan_weights(alphas: np.ndarray):
    """Replicates the reference scan's fp32 arithmetic exactly.

    Returns (wa, Ai, wb, Bi, ntot): per-step primary weight/slot, secondary
    (fire-only) weight/slot, and total fires per row.
    """
    a = np.ascontiguousarray(alphas, dtype=np.float32)
    Bb, Tt = a.shape
    ONE = np.float32(1.0)
    TH = np.float32(0.95)
    integrate = np.zeros(Bb, np.float32)
    n = np.zeros(Bb, np.int32)
    wa = np.empty((Bb, Tt), np.float32)
    wb = np.zeros((Bb, Tt), np.float32)
    Ai = np.empty((Bb, Tt), np.int32)
    Bi = np.empty((Bb, Tt), np.int32)
    for t in range(Tt):
        al = a[:, t]
        dist = ONE - integrate          # distribution_completion (fp32)
        integ = integrate + al          # fp32, same single add as reference
        f = integ > TH
        cur = np.where(f, dist, al)
        wa[:, t] = cur
        Ai[:, t] = n                    # n_prev
        wb[:, t] = np.where(f, al - cur, np.float32(0.0))
        Bi[:, t] = n + 1
        n = n + f
        integrate = np.where(f, integ - ONE, integ)  # exact subtract (Sterbenz)
    return wa, Ai, wb, Bi, n


def _build_scan_data(alphas: np.ndarray) -> np.ndarray:
    """Returns SD [B, KC, 4, NMM] fp16: (slot_a, w_a, slot_b, w_b) per
    (row p, matmul tile i), slot=DEAD when the contribution misses the
    tile's panel or is clipped."""
    wa, Ai, wb, Bi, ntot = _host_scan_weights(alphas)
    lim = np.minimum(ntot, L_OUT)[:, None].astype(np.int32)
    wa = np.where(Ai < lim, wa, np.float32(0.0))
    wb = np.where(Bi < lim, wb, np.float32(0.0))

    # panel-coverage asserts: every block's (nonzero) contributions must
    # fall inside the panels its matmuls cover.
    for bl, (t0, t1, S, panels) in enumerate(BLOCKS):
        pa = np.asarray(panels)
        for idx, w in ((Ai, wa), (Bi, wb)):
            sl = idx[:, t0:t1]
            live = w[:, t0:t1] != 0.0
            ok = np.isin(np.clip(sl, 0, LPAD - 1) // 128, pa) | ~live
            if not ok.all():
                raise AssertionError(f"block {bl} has mass outside panels {panels}")

    SD = np.zeros((B, KC, 4, NMM), np.float16)
    rows = np.arange(KC)
    for i, (bl, j, p) in enumerate(MMS):
        t0, t1, S, _ = BLOCKS[bl]
        t = t0 + S * rows + j                      # [KC]
        for c, (idx, w) in enumerate(((Ai, wa), (Bi, wb))):
            s = idx[:, t] - 128 * p                # [B, KC]
            valid = (s >= 0) & (s < 128) & (w[:, t] != 0.0)
            SD[:, :, 2 * c, i] = np.where(valid, s, DEAD).astype(np.float16)
            SD[:, :, 2 * c + 1, i] = np.where(valid, w[:, t], 0.0).astype(np.float16)
    return np.ascontiguousarray(SD)


def _build_program():
    """Builds + compiles the per-core Bass/Tile program (SPMD, shared)."""
    import concourse.bacc as bacc
    import concourse.mybir as mybir
    import concourse.tile as tile

    nc = bacc.Bacc(
        "TRN2",
        target_bir_lowering=False,
        debug=False,
        num_devices=N_CORES,
        dynamic_dma_scratch_size=65536,
    )
    f32 = mybir.dt.float32
    f16 = mybir.dt.float16
    eq = mybir.AluOpType.is_equal
    mul = mybir.AluOpType.mult
    add = mybir.AluOpType.add

    hid = nc.dram_tensor(
        "hidden_sh", [EX_PER_CORE, T, H], f16, kind="ExternalInput"
    )
    sdd = nc.dram_tensor(
        "sd_sh", [KC, EX_PER_CORE, 4, NMM], f16, kind="ExternalInput"
    )
    out = nc.dram_tensor(
        "out_sh", [EX_PER_CORE, L_OUT, H], f16, kind="ExternalOutput"
    )

    with tile.TileContext(nc) as tc:
        with (
            tc.tile_pool(name="hp0", bufs=4) as hpool0,    # b0 [125,7,H]
            tc.tile_pool(name="hp1", bufs=4) as hpool1,    # b1 [125,2,H]
            tc.tile_pool(name="hp2", bufs=4) as hpool2,    # b2 [125,7,H]
            tc.tile_pool(name="wp", bufs=4) as wpool,
            tc.tile_pool(name="aux", bufs=1) as apool,
            tc.tile_pool(name="dummy", bufs=2) as dpool,
            tc.tile_pool(name="ob", bufs=8) as opool,
            tc.tile_pool(name="psp", bufs=3, space="PSUM") as pspool,
            tc.tile_pool(name="pspd", bufs=1, space="PSUM") as pspool_d,
        ):
            # HAM warm-up fodder: zeroed operands, dedicated PSUM bank.
            dw = dpool.tile([KC, 128], f16)
            drh = dpool.tile([KC, H], f16)
            nc.vector.memset(dw[:], 0.0)
            nc.vector.memset(drh[:], 0.0)
            dps = pspool_d.tile([128, H], f32, tag="dummy")

            def dummy_mms(n):
                for _ in range(n):
                    nc.tensor.matmul(dps[:], dw[:], drh[:], start=True, stop=True)

            # iota ramp 0..127 repeated per matmul tile (values exact fp16)
            ia = apool.tile([KC, NMM, 128], f16)
            nc.gpsimd.iota(
                ia[:], pattern=[[0, NMM], [1, 128]], channel_multiplier=0,
                allow_small_or_imprecise_dtypes=True,
            )
            msk = apool.tile([KC, NMM, 128], f16)

            # ---- input DMAs, in exact rotation slot order ----
            hpools = {"b0": hpool0, "b1": hpool1, "b2": hpool2}
            htiles = [dict() for _ in range(EX_PER_CORE)]
            sdt = None
            for e, kind in SLOT_PLAN:
                if kind == "SD":
                    sdt = apool.tile([KC, EX_PER_CORE, 4, NMM], f16)
                    nc.gpsimd.dma_start(sdt[:], sdd[:, :, :, :])
                else:
                    bl = int(kind[1])
                    t0, t1, S, _ = BLOCKS[bl]
                    ht = hpools[kind].tile([KC, S, H], f16, name=kind)
                    src = hid[e, t0:t1, :].rearrange("(p j) h -> p j h", j=S)
                    nc.gpsimd.dma_start(ht[:], src)
                    htiles[e][kind] = ht

            # ---- on-device W expansion (vector engine) ----
            wtiles = []
            shp = [KC, NMM, 128]
            for e in range(EX_PER_CORE):
                sa = sdt[:, e, 0, :].broadcast_to(shp)
                wa = sdt[:, e, 1, :].broadcast_to(shp)
                sb = sdt[:, e, 2, :].broadcast_to(shp)
                wb = sdt[:, e, 3, :].broadcast_to(shp)
                wt = wpool.tile(shp, f16)
                nc.vector.tensor_tensor(wt[:], ia[:], sa, op=eq)
                nc.vector.tensor_tensor(wt[:], wt[:], wa, op=mul)
                nc.vector.tensor_tensor(msk[:], ia[:], sb, op=eq)
                nc.vector.tensor_tensor(msk[:], msk[:], wb, op=mul)
                nc.vector.tensor_tensor(wt[:], wt[:], msk[:], op=add)
                wtiles.append(wt)

            def rhs(e, bl, j):
                return htiles[e][f"b{bl}"][:, j, :]

            # ---- matmul + copy-out pipeline ----
            last_i = {p: max(i for i, m in enumerate(MMS) if m[2] == p) for p in (0, 1)}
            dummy_mms(N_WARM)
            for e in range(EX_PER_CORE):
                if e:
                    dummy_mms(N_KEEP)
                wt = wtiles[e]
                panels = [
                    pspool.tile([128, H], f32, name=f"panel{p}", tag=f"panel{p}")
                    for p in range(2)
                ]
                first = [True, True]
                for i, (bl, j, p) in enumerate(MMS):
                    nc.tensor.matmul(
                        panels[p][:], wt[:, i, :], rhs(e, bl, j),
                        start=first[p], stop=(i == last_i[p]),
                    )
                    first[p] = False
                ob0 = opool.tile([128, H], f16)
                nc.vector.tensor_copy(ob0[:], panels[0][:])
                nc.gpsimd.dma_start(out[e, 0:128, :], ob0[:])
                ob1 = opool.tile([128, H], f16)
                nc.scalar.copy(ob1[0 : L_OUT - 128, :], panels[1][0 : L_OUT - 128, :])
                nc.gpsimd.dma_start(out[e, 128:L_OUT, :], ob1[0 : L_OUT - 128, :])
    nc.compile()
    return nc


def kernel(hidden: np.ndarray, alphas: np.ndarray) -> np.ndarray:
    global _PROGRAM, LAST_RESULT
    from concourse.bass_utils import run_bass_kernel_spmd

    hidden = np.ascontiguousarray(np.asarray(hidden), dtype=np.float32)
    alphas = np.ascontiguousarray(np.asarray(alphas), dtype=np.float32)
    assert hidden.shape == (B, T, H) and alphas.shape == (B, T)

    hidden16 = hidden.astype(np.float16)
    SD = _build_scan_data(alphas)          # [B, KC, 4, NMM]

    if _PROGRAM is None:
        _PROGRAM = _build_program()
    nc = _PROGRAM

    in_maps = [
        {
            "hidden_sh": hidden16[i * EX_PER_CORE : (i + 1) * EX_PER_CORE],
            "sd_sh": np.ascontiguousarray(
                SD[i * EX_PER_CORE : (i + 1) * EX_PER_CORE].transpose(1, 0, 2, 3)
            ),
        }
        for i in range(N_CORES)
    ]
    res = run_bass_kernel_spmd(nc, in_maps, list(range(N_CORES)), **RUN_KWARGS)
    LAST_RESULT = res
    out16 = np.concatenate([r["out_sh"] for r in res.results], axis=0)
    return out16.astype(np.float32)
